# Initial kernel scaffold
#
"""MLA prefill attention kernel for Trainium2 (Bass/Tile), SPMD over 8 cores.

Problem (see reference): B=2, S=2048, H=16, QK=192 (128 nope + 64 rope), VD=128.
  q            [B*S, H*192]
  latent_vec_k [B*S, 512+64]
  W_kv_b       [512, H*256]
  out          [B*S, H*128]

Sharding: core c handles batch b = c//4 and heads [4g, 4g+4) with g = c%4.
latent for the core's batch is replicated across its 4 head-group cores.

Per-core device program (all matmuls on PE, softmax exp on ACT):
  kT[h]  [128, S]  = (W_k[:, h].T @ latT-chunks)          (fp32r matmuls)
  v      [S, 4*128] = latT-chunks.T @ W_v, stored bf16 with a ones column
                      appended per head (softmax denominator rides the PV matmul)
  scoresT[k, qb] [128, 512] = kT/k_peT.T @ qT  (fp32r, causal block-skipped)
  PT = exp(SCALE * scoresT) via ACT, bf16, diagonal 128x128 blocks masked
  outT-natural: out[sq 128, 129] = sum_k PT[k].T @ v_aug[k]  (bf16 matmuls)
  out = out[:, :128] * 1/out[:, 128]  (DVE), DMA to HBM.
"""

import sys

if "/opt/trn_rl_repo" not in sys.path:
    sys.path.insert(0, "/opt/trn_rl_repo")

from contextlib import ExitStack

import ml_dtypes
import numpy as np

import concourse.bass as bass
import concourse.mybir as mybir
import concourse.tile as tile
from concourse import bass_utils

B, S, H = 2, 2048, 16
KV_RANK, ROPE, NOPE, VD = 512, 64, 128, 128
QK = NOPE + ROPE
SCALE = 1.0 / float(np.sqrt(QK))

NCORES = 8
HL = 4            # heads per core
NBG = NCORES // B  # head-group cores per batch
NK = S // 128     # 16 key chunks of 128
NQB = S // 512    # 4 query blocks of 512

F32 = mybir.dt.float32
F32R = mybir.dt.float32r
BF16 = mybir.dt.bfloat16
EXP = mybir.ActivationFunctionType.Exp


def _build_program():
    nc = bass.Bass("TRN2", target_bir_lowering=False, debug=False)

    qtn = nc.dram_tensor("qtn", [HL, NOPE, S], F32, kind="ExternalInput").ap()
    qtr = nc.dram_tensor("qtr", [HL, ROPE, S], F32, kind="ExternalInput").ap()
    latt = nc.dram_tensor("latt", [KV_RANK + ROPE, S], F32, kind="ExternalInput").ap()
    wk = nc.dram_tensor("wk", [KV_RANK, HL * NOPE], F32, kind="ExternalInput").ap()
    wv = nc.dram_tensor("wv", [KV_RANK, HL * VD], F32, kind="ExternalInput").ap()
    tri = nc.dram_tensor("tri", [128, 128], BF16, kind="ExternalInput").ap()
    out = nc.dram_tensor("out", [S, HL * VD], F32, kind="ExternalOutput").ap()

    NR = KV_RANK // 128  # 4 latent chunks

    with ExitStack() as ctx:
        tc = ctx.enter_context(tile.TileContext(nc))
        cpool = ctx.enter_context(tc.tile_pool(name="cpool", bufs=1))
        ktp = ctx.enter_context(tc.tile_pool(name="ktp", bufs=2))
        ptp = ctx.enter_context(tc.tile_pool(name="ptp", bufs=12))
        smal = ctx.enter_context(tc.tile_pool(name="smal", bufs=4))
        outp = ctx.enter_context(tc.tile_pool(name="outp", bufs=4))
        ps_proj = ctx.enter_context(tc.tile_pool(name="ps_proj", bufs=2, space="PSUM"))
        ps_sc = ctx.enter_context(tc.tile_pool(name="ps_sc", bufs=2, space="PSUM"))
        ps_pv = ctx.enter_context(tc.tile_pool(name="ps_pv", bufs=2, space="PSUM"))

        # ---- constants / inputs into SBUF ----
        lat_sb = []
        for r in range(NR):
            t = cpool.tile([128, S], F32, name=f"lat{r}", tag=f"lat{r}")
            nc.sync.dma_start(out=t, in_=latt[r * 128:(r + 1) * 128, :])
            lat_sb.append(t)
        rope_sb = cpool.tile([ROPE, S], F32, name="rope", tag="rope")
        nc.sync.dma_start(out=rope_sb, in_=latt[KV_RANK:KV_RANK + ROPE, :])

        wk_sb, wv_sb = [], []
        for r in range(NR):
            t = cpool.tile([128, HL * NOPE], F32, name=f"wk{r}", tag=f"wk{r}")
            nc.sync.dma_start(out=t, in_=wk[r * 128:(r + 1) * 128, :])
            wk_sb.append(t)
            t2 = cpool.tile([128, HL * VD], F32, name=f"wv{r}", tag=f"wv{r}")
            nc.sync.dma_start(out=t2, in_=wv[r * 128:(r + 1) * 128, :])
            wv_sb.append(t2)

        tri_sb = cpool.tile([128, 128], BF16, name="tri", tag="tri")
        nc.sync.dma_start(out=tri_sb, in_=tri)

        qn_sb, qr_sb = [], []
        for h in range(HL):
            t = cpool.tile([NOPE, S], F32, name=f"qn{h}", tag=f"qn{h}")
            nc.sync.dma_start(out=t, in_=qtn[h])
            qn_sb.append(t)
            t2 = cpool.tile([ROPE, S], F32, name=f"qr{h}", tag=f"qr{h}")
            nc.sync.dma_start(out=t2, in_=qtr[h])
            qr_sb.append(t2)

        # ---- v projection: v[t, h*129+d], bf16, ones column per head ----
        v_sb = []
        for c in range(NK):
            ps = ps_proj.tile([128, HL * VD], F32, name="psv", tag="proj")
            for r in range(NR):
                nc.tensor.matmul(
                    ps,
                    lhsT=lat_sb[r][:, c * 128:(c + 1) * 128].bitcast(F32R),
                    rhs=wv_sb[r].bitcast(F32R),
                    start=(r == 0),
                    stop=(r == NR - 1),
                )
            vt = cpool.tile([128, HL * (VD + 1)], BF16, name=f"v{c}", tag=f"v{c}")
            v3 = vt.rearrange("p (h d) -> p h d", d=VD + 1)
            nc.vector.tensor_copy(
                out=v3[:, :, 0:VD],
                in_=ps.rearrange("p (h d) -> p h d", d=VD),
            )
            nc.vector.memset(v3[:, :, VD:VD + 1], 1.0)
            v_sb.append(vt)

        # ---- per head: kT projection, scores, softmax, PV ----
        for h in range(HL):
            kt = ktp.tile([128, S], F32, name=f"kt{h}", tag="kt")
            for tb in range(S // 512):
                ps = ps_proj.tile([128, 512], F32, name="psk", tag="proj")
                for r in range(NR):
                    nc.tensor.matmul(
                        ps,
                        lhsT=wk_sb[r][:, h * NOPE:(h + 1) * NOPE].bitcast(F32R),
                        rhs=lat_sb[r][:, tb * 512:(tb + 1) * 512].bitcast(F32R),
                        start=(r == 0),
                        stop=(r == NR - 1),
                    )
                nc.vector.tensor_copy(out=kt[:, tb * 512:(tb + 1) * 512], in_=ps)

            for qb in range(NQB):
                nk = 4 * qb + 4  # causal: key chunks 0..nk-1 for this q block
                pt_tiles = []
                for kp in range(nk // 2):
                    ps = ps_sc.tile([128, 1024], F32, name="pss", tag="sc")
                    for half in range(2):
                        k = 2 * kp + half
                        dst = ps[:, half * 512:(half + 1) * 512]
                        nc.tensor.matmul(
                            dst,
                            lhsT=kt[:, k * 128:(k + 1) * 128].bitcast(F32R),
                            rhs=qn_sb[h][:, qb * 512:(qb + 1) * 512].bitcast(F32R),
                            start=True,
                            stop=False,
                        )
                        nc.tensor.matmul(
                            dst,
                            lhsT=rope_sb[:, k * 128:(k + 1) * 128].bitcast(F32R),
                            rhs=qr_sb[h][:, qb * 512:(qb + 1) * 512].bitcast(F32R),
                            start=False,
                            stop=True,
                        )
                    pt = ptp.tile([128, 1024], BF16, name="pt", tag="pt")
                    nc.scalar.activation(out=pt, in_=ps, func=EXP, scale=SCALE)
                    pt_tiles.append(pt)

                # mask the diagonal 128x128 blocks (the only partially-valid
                # blocks the PV accumulation below will consume)
                for c in range(4):
                    k = 4 * qb + c
                    sl = pt_tiles[k // 2][:, (k % 2) * 512 + c * 128:(k % 2) * 512 + c * 128 + 128]
                    nc.vector.tensor_mul(sl, sl, tri_sb)

                for c in range(4):
                    qi = 4 * qb + c  # 128-row output tile index
                    po = ps_pv.tile([128, VD + 1], F32, name="po", tag="pv")
                    for k in range(qi + 1):
                        lhs = pt_tiles[k // 2][:, (k % 2) * 512 + c * 128:(k % 2) * 512 + c * 128 + 128]
                        nc.tensor.matmul(
                            po,
                            lhsT=lhs,
                            rhs=v_sb[k][:, h * (VD + 1):(h + 1) * (VD + 1)],
                            start=(k == 0),
                            stop=(k == qi),
                        )
                    rc = smal.tile([128, 1], F32, name="rc", tag="rc")
                    nc.vector.reciprocal(rc, po[:, VD:VD + 1])
                    ot = outp.tile([128, VD], F32, name="ot", tag="ot")
                    nc.vector.tensor_scalar_mul(ot, po[:, 0:VD], rc)
                    nc.sync.dma_start(
                        out=out[qi * 128:(qi + 1) * 128, h * VD:(h + 1) * VD],
                        in_=ot,
                    )
    return nc


_NC_CACHE = None


def _get_nc():
    global _NC_CACHE
    if _NC_CACHE is None:
        _NC_CACHE = _build_program()
    return _NC_CACHE


def _shard_inputs(q, latent_vec_k, W_kv_b):
    q4 = np.asarray(q, np.float32).reshape(B, S, H, QK)
    lat = np.asarray(latent_vec_k, np.float32)
    W4 = np.asarray(W_kv_b, np.float32).reshape(KV_RANK, H, NOPE + VD)
    tri = np.triu(np.ones((128, 128), np.float32)).astype(ml_dtypes.bfloat16)

    in_maps = []
    for c in range(NCORES):
        b, g = divmod(c, NBG)
        hg = slice(HL * g, HL * g + HL)
        qh = q4[b, :, hg, :]  # [S, HL, QK]
        in_maps.append({
            "qtn": np.ascontiguousarray(qh[:, :, :NOPE].transpose(1, 2, 0)),
            "qtr": np.ascontiguousarray(qh[:, :, NOPE:].transpose(1, 2, 0)),
            "latt": np.ascontiguousarray(lat[b * S:(b + 1) * S, :].T),
            "wk": np.ascontiguousarray(W4[:, hg, :NOPE].reshape(KV_RANK, HL * NOPE)),
            "wv": np.ascontiguousarray(W4[:, hg, NOPE:].reshape(KV_RANK, HL * VD)),
            "tri": tri,
        })
    return in_maps


def run_cores(q, latent_vec_k, W_kv_b, trace=False, **kw):
    nc = _get_nc()
    in_maps = _shard_inputs(q, latent_vec_k, W_kv_b)
    res = bass_utils.run_bass_kernel_spmd(
        nc, in_maps, core_ids=list(range(NCORES)), trace=trace, **kw
    )
    full = np.empty((B * S, H * VD), np.float32)
    for c in range(NCORES):
        b, g = divmod(c, NBG)
        full[b * S:(b + 1) * S, g * HL * VD:(g + 1) * HL * VD] = res.results[c]["out"]
    return full, res


def kernel(q, latent_vec_k, W_kv_b):
    return run_cores(q, latent_vec_k, W_kv_b)[0]


# revision 4
# speedup vs baseline: 1.2201x; 1.2201x over previous
"""MLA prefill attention kernel for Trainium2 (Bass/Tile), SPMD over 8 cores.

Problem (see reference): B=2, S=2048, H=16, QK=192 (128 nope + 64 rope), VD=128.
  q            [B*S, H*192]
  latent_vec_k [B*S, 512+64]
  W_kv_b       [512, H*256]
  out          [B*S, H*128]

Sharding: core c handles batch b = c//4 and heads [4g, 4g+4) with g = c%4.
latent for the core's batch is replicated across its 4 head-group cores.

Per-core device program (all matmuls on PE, softmax exp on ACT):
  kT[h]  [128, S]  = (W_k[:, h].T @ latT-chunks)          (fp32r matmuls)
  v      [S, 4*128] = latT-chunks.T @ W_v, stored bf16 with a ones column
                      appended per head (softmax denominator rides the PV matmul)
  scoresT[k, qb] [128, 512] = kT/k_peT.T @ qT  (fp32r, causal block-skipped)
  PT = exp(SCALE * scoresT) via ACT, bf16, diagonal 128x128 blocks masked
  outT-natural: out[sq 128, 129] = sum_k PT[k].T @ v_aug[k]  (bf16 matmuls)
  out = out[:, :128] * 1/out[:, 128]  (DVE), DMA to HBM.
"""

import sys

if "/opt/trn_rl_repo" not in sys.path:
    sys.path.insert(0, "/opt/trn_rl_repo")

from contextlib import ExitStack

import ml_dtypes
import numpy as np

import concourse.bass as bass
import concourse.bacc as bacc
import concourse.mybir as mybir
import concourse.tile as tile
from concourse import bass_utils

B, S, H = 2, 2048, 16
KV_RANK, ROPE, NOPE, VD = 512, 64, 128, 128
QK = NOPE + ROPE
SCALE = 1.0 / float(np.sqrt(QK))

NCORES = 8
HL = 4            # heads per core
NBG = NCORES // B  # head-group cores per batch
NK = S // 128     # 16 key chunks of 128
NQB = S // 512    # 4 query blocks of 512

F32 = mybir.dt.float32
F32R = mybir.dt.float32r
F16 = mybir.dt.float16
BF16 = mybir.dt.bfloat16
EXP = mybir.ActivationFunctionType.Exp


def _build_program():
    nc = bacc.Bacc("TRN2", target_bir_lowering=False, debug=False)

    qtn = nc.dram_tensor("qtn", [HL, NOPE, S], F16, kind="ExternalInput").ap()
    qtr = nc.dram_tensor("qtr", [HL, ROPE, S], F16, kind="ExternalInput").ap()
    latt = nc.dram_tensor("latt", [KV_RANK + ROPE, S], F16, kind="ExternalInput").ap()
    wk = nc.dram_tensor("wk", [KV_RANK, HL * NOPE], F16, kind="ExternalInput").ap()
    wv = nc.dram_tensor("wv", [KV_RANK, HL * VD], F16, kind="ExternalInput").ap()
    tri = nc.dram_tensor("tri", [128, 128], F16, kind="ExternalInput").ap()
    out = nc.dram_tensor("out", [S, HL * VD], F32, kind="ExternalOutput").ap()

    NR = KV_RANK // 128  # 4 latent chunks

    with ExitStack() as ctx:
        tc = ctx.enter_context(tile.TileContext(nc))
        cpool = ctx.enter_context(tc.tile_pool(name="cpool", bufs=1))
        ktp = ctx.enter_context(tc.tile_pool(name="ktp", bufs=2))
        ptp = ctx.enter_context(tc.tile_pool(name="ptp", bufs=12))
        smal = ctx.enter_context(tc.tile_pool(name="smal", bufs=4))
        outp = ctx.enter_context(tc.tile_pool(name="outp", bufs=4))
        ps_proj = ctx.enter_context(tc.tile_pool(name="ps_proj", bufs=2, space="PSUM"))
        ps_sc = ctx.enter_context(tc.tile_pool(name="ps_sc", bufs=2, space="PSUM"))
        ps_pv = ctx.enter_context(tc.tile_pool(name="ps_pv", bufs=2, space="PSUM"))

        # ---- constants / inputs into SBUF ----
        lat_sb = []
        for r in range(NR):
            t = cpool.tile([128, S], F16, name=f"lat{r}", tag=f"lat{r}")
            nc.sync.dma_start(out=t, in_=latt[r * 128:(r + 1) * 128, :])
            lat_sb.append(t)
        rope_sb = cpool.tile([ROPE, S], F16, name="rope", tag="rope")
        nc.sync.dma_start(out=rope_sb, in_=latt[KV_RANK:KV_RANK + ROPE, :])

        wk_sb, wv_sb = [], []
        for r in range(NR):
            t = cpool.tile([128, HL * NOPE], F16, name=f"wk{r}", tag=f"wk{r}")
            nc.sync.dma_start(out=t, in_=wk[r * 128:(r + 1) * 128, :])
            wk_sb.append(t)
            t2 = cpool.tile([128, HL * VD], F16, name=f"wv{r}", tag=f"wv{r}")
            nc.sync.dma_start(out=t2, in_=wv[r * 128:(r + 1) * 128, :])
            wv_sb.append(t2)

        tri_sb = cpool.tile([128, 128], F16, name="tri", tag="tri")
        nc.sync.dma_start(out=tri_sb, in_=tri)

        qn_sb, qr_sb = [], []
        for h in range(HL):
            t = cpool.tile([NOPE, S], F16, name=f"qn{h}", tag=f"qn{h}")
            nc.sync.dma_start(out=t, in_=qtn[h])
            qn_sb.append(t)
            t2 = cpool.tile([ROPE, S], F16, name=f"qr{h}", tag=f"qr{h}")
            nc.sync.dma_start(out=t2, in_=qtr[h])
            qr_sb.append(t2)

        # ---- v projection: v[t, h*129+d], bf16, ones column per head ----
        v_sb = []
        for c in range(NK):
            ps = ps_proj.tile([128, HL * VD], F32, name="psv", tag="proj")
            for r in range(NR):
                nc.tensor.matmul(
                    ps,
                    lhsT=lat_sb[r][:, c * 128:(c + 1) * 128],
                    rhs=wv_sb[r],
                    start=(r == 0),
                    stop=(r == NR - 1),
                )
            vt = cpool.tile([128, HL * (VD + 1)], F16, name=f"v{c}", tag=f"v{c}")
            v3 = vt.rearrange("p (h d) -> p h d", d=VD + 1)
            nc.vector.tensor_copy(
                out=v3[:, :, 0:VD],
                in_=ps.rearrange("p (h d) -> p h d", d=VD),
            )
            nc.vector.memset(v3[:, :, VD:VD + 1], 1.0)
            v_sb.append(vt)

        # ---- per head: kT projection, scores, softmax, PV ----
        for h in range(HL):
            kt = ktp.tile([128, S], F16, name=f"kt{h}", tag="kt")
            for tb in range(S // 512):
                ps = ps_proj.tile([128, 512], F32, name="psk", tag="proj")
                for r in range(NR):
                    nc.tensor.matmul(
                        ps,
                        lhsT=wk_sb[r][:, h * NOPE:(h + 1) * NOPE],
                        rhs=lat_sb[r][:, tb * 512:(tb + 1) * 512],
                        start=(r == 0),
                        stop=(r == NR - 1),
                    )
                nc.vector.tensor_copy(out=kt[:, tb * 512:(tb + 1) * 512], in_=ps)

            for qb in range(NQB):
                nk = 4 * qb + 4  # causal: key chunks 0..nk-1 for this q block
                pt_tiles = []
                for kp in range(nk // 2):
                    ps = ps_sc.tile([128, 1024], F32, name="pss", tag="sc")
                    for half in range(2):
                        k = 2 * kp + half
                        dst = ps[:, half * 512:(half + 1) * 512]
                        nc.tensor.matmul(
                            dst,
                            lhsT=kt[:, k * 128:(k + 1) * 128],
                            rhs=qn_sb[h][:, qb * 512:(qb + 1) * 512],
                            start=True,
                            stop=False,
                        )
                        nc.tensor.matmul(
                            dst,
                            lhsT=rope_sb[:, k * 128:(k + 1) * 128],
                            rhs=qr_sb[h][:, qb * 512:(qb + 1) * 512],
                            start=False,
                            stop=True,
                        )
                    pt = ptp.tile([128, 1024], F16, name="pt", tag="pt")
                    nc.scalar.activation(out=pt, in_=ps, func=EXP, scale=SCALE)
                    pt_tiles.append(pt)

                # mask the diagonal 128x128 blocks (the only partially-valid
                # blocks the PV accumulation below will consume)
                for c in range(4):
                    k = 4 * qb + c
                    sl = pt_tiles[k // 2][:, (k % 2) * 512 + c * 128:(k % 2) * 512 + c * 128 + 128]
                    nc.vector.tensor_mul(sl, sl, tri_sb)

                for c in range(4):
                    qi = 4 * qb + c  # 128-row output tile index
                    po = ps_pv.tile([128, VD + 1], F32, name="po", tag="pv")
                    for k in range(qi + 1):
                        lhs = pt_tiles[k // 2][:, (k % 2) * 512 + c * 128:(k % 2) * 512 + c * 128 + 128]
                        nc.tensor.matmul(
                            po,
                            lhsT=lhs,
                            rhs=v_sb[k][:, h * (VD + 1):(h + 1) * (VD + 1)],
                            start=(k == 0),
                            stop=(k == qi),
                        )
                    rc = smal.tile([128, 1], F32, name="rc", tag="rc")
                    nc.vector.reciprocal(rc, po[:, VD:VD + 1])
                    ot = outp.tile([128, VD], F32, name="ot", tag="ot")
                    nc.vector.tensor_scalar_mul(ot, po[:, 0:VD], rc)
                    nc.sync.dma_start(
                        out=out[qi * 128:(qi + 1) * 128, h * VD:(h + 1) * VD],
                        in_=ot,
                    )
    nc.compile()
    return nc


_NC_CACHE = None


def _get_nc():
    global _NC_CACHE
    if _NC_CACHE is None:
        _NC_CACHE = _build_program()
    return _NC_CACHE


def _shard_inputs(q, latent_vec_k, W_kv_b):
    q4 = np.asarray(q, np.float32).reshape(B, S, H, QK)
    lat = np.asarray(latent_vec_k, np.float32)
    W4 = np.asarray(W_kv_b, np.float32).reshape(KV_RANK, H, NOPE + VD)
    tri = np.triu(np.ones((128, 128), np.float32)).astype(np.float16)

    in_maps = []
    for c in range(NCORES):
        b, g = divmod(c, NBG)
        hg = slice(HL * g, HL * g + HL)
        qh = q4[b, :, hg, :]  # [S, HL, QK]
        in_maps.append({
            "qtn": np.ascontiguousarray(qh[:, :, :NOPE].transpose(1, 2, 0)).astype(np.float16),
            "qtr": np.ascontiguousarray(qh[:, :, NOPE:].transpose(1, 2, 0)).astype(np.float16),
            "latt": np.ascontiguousarray(lat[b * S:(b + 1) * S, :].T).astype(np.float16),
            "wk": W4[:, hg, :NOPE].reshape(KV_RANK, HL * NOPE).astype(np.float16),
            "wv": W4[:, hg, NOPE:].reshape(KV_RANK, HL * VD).astype(np.float16),
            "tri": tri,
        })
    return in_maps


def run_cores(q, latent_vec_k, W_kv_b, trace=False, **kw):
    nc = _get_nc()
    in_maps = _shard_inputs(q, latent_vec_k, W_kv_b)
    res = bass_utils.run_bass_kernel_spmd(
        nc, in_maps, core_ids=list(range(NCORES)), trace=trace, **kw
    )
    full = np.empty((B * S, H * VD), np.float32)
    for c in range(NCORES):
        b, g = divmod(c, NBG)
        full[b * S:(b + 1) * S, g * HL * VD:(g + 1) * HL * VD] = res.results[c]["out"]
    return full, res


def kernel(q, latent_vec_k, W_kv_b):
    return run_cores(q, latent_vec_k, W_kv_b)[0]


# revision 7
# speedup vs baseline: 1.2463x; 1.0215x over previous
"""MLA prefill attention kernel for Trainium2 (Bass/Tile), SPMD over 8 cores.

Problem (see reference): B=2, S=2048, H=16, QK=192 (128 nope + 64 rope), VD=128.
  q            [B*S, H*192]
  latent_vec_k [B*S, 512+64]
  W_kv_b       [512, H*256]
  out          [B*S, H*128]

Sharding: core c handles batch b = c//4 and heads [4g, 4g+4) with g = c%4.
latent for the core's batch is replicated across its 4 head-group cores.

Per-core device program (all matmuls on PE, softmax exp on ACT):
  kT[h]  [128, S]  = (W_k[:, h].T @ latT-chunks)          (fp32r matmuls)
  v      [S, 4*128] = latT-chunks.T @ W_v, stored bf16 with a ones column
                      appended per head (softmax denominator rides the PV matmul)
  scoresT[k, qb] [128, 512] = kT/k_peT.T @ qT  (fp32r, causal block-skipped)
  PT = exp(SCALE * scoresT) via ACT, bf16, diagonal 128x128 blocks masked
  outT-natural: out[sq 128, 129] = sum_k PT[k].T @ v_aug[k]  (bf16 matmuls)
  out = out[:, :128] * 1/out[:, 128]  (DVE), DMA to HBM.
"""

import sys

if "/opt/trn_rl_repo" not in sys.path:
    sys.path.insert(0, "/opt/trn_rl_repo")

from contextlib import ExitStack

import ml_dtypes
import numpy as np

import concourse.bass as bass
import concourse.bacc as bacc
import concourse.mybir as mybir
import concourse.tile as tile
from concourse import bass_utils

B, S, H = 2, 2048, 16
KV_RANK, ROPE, NOPE, VD = 512, 64, 128, 128
QK = NOPE + ROPE
SCALE = 1.0 / float(np.sqrt(QK))

NCORES = 8
HL = 4            # heads per core
NBG = NCORES // B  # head-group cores per batch
NK = S // 128     # 16 key chunks of 128
NQB = S // 512    # 4 query blocks of 512

F32 = mybir.dt.float32
F32R = mybir.dt.float32r
F16 = mybir.dt.float16
BF16 = mybir.dt.bfloat16
EXP = mybir.ActivationFunctionType.Exp


def _build_program():
    nc = bacc.Bacc("TRN2", target_bir_lowering=False, debug=False)

    qtn = nc.dram_tensor("qtn", [HL, NOPE, S], F16, kind="ExternalInput").ap()
    qtr = nc.dram_tensor("qtr", [HL, ROPE, S], F16, kind="ExternalInput").ap()
    latt = nc.dram_tensor("latt", [KV_RANK + ROPE, S], F16, kind="ExternalInput").ap()
    wk = nc.dram_tensor("wk", [KV_RANK, HL * NOPE], F16, kind="ExternalInput").ap()
    wv = nc.dram_tensor("wv", [KV_RANK, HL * VD], F16, kind="ExternalInput").ap()
    tri = nc.dram_tensor("tri", [128, 128], F16, kind="ExternalInput").ap()
    out = nc.dram_tensor("out", [S, HL * VD], F32, kind="ExternalOutput").ap()

    NR = KV_RANK // 128  # 4 latent chunks

    with ExitStack() as ctx:
        tc = ctx.enter_context(tile.TileContext(nc))
        cpool = ctx.enter_context(tc.tile_pool(name="cpool", bufs=1))
        ktp = ctx.enter_context(tc.tile_pool(name="ktp", bufs=2))
        ptp = ctx.enter_context(tc.tile_pool(name="ptp", bufs=16))
        smal = ctx.enter_context(tc.tile_pool(name="smal", bufs=4))
        outp = ctx.enter_context(tc.tile_pool(name="outp", bufs=4))
        ps_proj = ctx.enter_context(tc.tile_pool(name="ps_proj", bufs=2, space="PSUM"))
        ps_sc = ctx.enter_context(tc.tile_pool(name="ps_sc", bufs=2, space="PSUM"))
        ps_pv = ctx.enter_context(tc.tile_pool(name="ps_pv", bufs=2, space="PSUM"))

        # ---- constants / inputs into SBUF ----
        lat_sb = []
        for r in range(NR):
            t = cpool.tile([128, S], F16, name=f"lat{r}", tag=f"lat{r}")
            # split halves so the first projection matmuls can start early
            nc.sync.dma_start(out=t[:, 0:S // 2], in_=latt[r * 128:(r + 1) * 128, 0:S // 2])
            nc.sync.dma_start(out=t[:, S // 2:S], in_=latt[r * 128:(r + 1) * 128, S // 2:S])
            lat_sb.append(t)
        rope_sb = cpool.tile([ROPE, S], F16, name="rope", tag="rope")
        nc.sync.dma_start(out=rope_sb, in_=latt[KV_RANK:KV_RANK + ROPE, :])

        wk_sb, wv_sb = [], []
        for r in range(NR):
            t = cpool.tile([128, HL * NOPE], F16, name=f"wk{r}", tag=f"wk{r}")
            nc.sync.dma_start(out=t, in_=wk[r * 128:(r + 1) * 128, :])
            wk_sb.append(t)
            t2 = cpool.tile([128, HL * VD], F16, name=f"wv{r}", tag=f"wv{r}")
            nc.sync.dma_start(out=t2, in_=wv[r * 128:(r + 1) * 128, :])
            wv_sb.append(t2)

        tri_sb = cpool.tile([128, 128], F16, name="tri", tag="tri")
        nc.sync.dma_start(out=tri_sb, in_=tri)

        qn_sb, qr_sb = [], []
        for h in range(HL):
            t = cpool.tile([NOPE, S], F16, name=f"qn{h}", tag=f"qn{h}")
            nc.sync.dma_start(out=t, in_=qtn[h])
            qn_sb.append(t)
            t2 = cpool.tile([ROPE, S], F16, name=f"qr{h}", tag=f"qr{h}")
            nc.sync.dma_start(out=t2, in_=qtr[h])
            qr_sb.append(t2)

        # ---- v projection: v[t, h*129+d], bf16, ones column per head ----
        v_sb = []
        for c in range(NK):
            ps = ps_proj.tile([128, HL * VD], F32, name="psv", tag="proj")
            for r in range(NR):
                nc.tensor.matmul(
                    ps,
                    lhsT=lat_sb[r][:, c * 128:(c + 1) * 128],
                    rhs=wv_sb[r],
                    start=(r == 0),
                    stop=(r == NR - 1),
                )
            vt = cpool.tile([128, HL * (VD + 1)], F16, name=f"v{c}", tag=f"v{c}")
            v3 = vt.rearrange("p (h d) -> p h d", d=VD + 1)
            nc.vector.tensor_copy(
                out=v3[:, :, 0:VD],
                in_=ps.rearrange("p (h d) -> p h d", d=VD),
            )
            nc.vector.memset(v3[:, :, VD:VD + 1], 1.0)
            v_sb.append(vt)

        # ---- kT projection for all heads (fills the input-DMA window) ----
        kt_all = []
        for h in range(HL):
            kt = ktp.tile([128, S], F16, name=f"kt{h}", tag=f"kt{h}")
            for tb in range(S // 512):
                ps = ps_proj.tile([128, 512], F32, name="psk", tag="proj")
                for r in range(NR):
                    nc.tensor.matmul(
                        ps,
                        lhsT=wk_sb[r][:, h * NOPE:(h + 1) * NOPE],
                        rhs=lat_sb[r][:, tb * 512:(tb + 1) * 512],
                        start=(r == 0),
                        stop=(r == NR - 1),
                    )
                nc.vector.tensor_copy(out=kt[:, tb * 512:(tb + 1) * 512], in_=ps)
            kt_all.append(kt)

        # ---- per head: scores, softmax, PV ----
        for h in range(HL):
            kt = kt_all[h]
            for qb in range(NQB):
                nk = 4 * qb + 4  # causal: key chunks 0..nk-1 for this q block
                pt_tiles = []
                for kp in range(nk // 2):
                    ps = ps_sc.tile([128, 1024], F32, name="pss", tag="sc")
                    for half in range(2):
                        k = 2 * kp + half
                        # diagonal chunks: only s_q >= 128c is unmasked, so
                        # trim the moving operand to the valid column range
                        c = k - 4 * qb
                        off = 128 * c if c > 0 else 0
                        dst = ps[:, half * 512 + off:(half + 1) * 512]
                        nc.tensor.matmul(
                            dst,
                            lhsT=kt[:, k * 128:(k + 1) * 128],
                            rhs=qn_sb[h][:, qb * 512 + off:(qb + 1) * 512],
                            start=True,
                            stop=False,
                        )
                        nc.tensor.matmul(
                            dst,
                            lhsT=rope_sb[:, k * 128:(k + 1) * 128],
                            rhs=qr_sb[h][:, qb * 512 + off:(qb + 1) * 512],
                            start=False,
                            stop=True,
                        )
                    pt = ptp.tile([128, 1024], F16, name="pt", tag="pt")
                    nc.scalar.activation(out=pt, in_=ps, func=EXP, scale=SCALE)
                    pt_tiles.append(pt)

                # mask the diagonal 128x128 blocks (the only partially-valid
                # blocks the PV accumulation below will consume)
                for c in range(4):
                    k = 4 * qb + c
                    sl = pt_tiles[k // 2][:, (k % 2) * 512 + c * 128:(k % 2) * 512 + c * 128 + 128]
                    nc.vector.tensor_mul(sl, sl, tri_sb)

                for c in range(4):
                    qi = 4 * qb + c  # 128-row output tile index
                    po = ps_pv.tile([128, VD + 1], F32, name="po", tag="pv")
                    for k in range(qi + 1):
                        lhs = pt_tiles[k // 2][:, (k % 2) * 512 + c * 128:(k % 2) * 512 + c * 128 + 128]
                        nc.tensor.matmul(
                            po,
                            lhsT=lhs,
                            rhs=v_sb[k][:, h * (VD + 1):(h + 1) * (VD + 1)],
                            start=(k == 0),
                            stop=(k == qi),
                        )
                    rc = smal.tile([128, 1], F32, name="rc", tag="rc")
                    nc.vector.reciprocal(rc, po[:, VD:VD + 1])
                    ot = outp.tile([128, VD], F32, name="ot", tag="ot")
                    nc.vector.tensor_scalar_mul(ot, po[:, 0:VD], rc)
                    nc.sync.dma_start(
                        out=out[qi * 128:(qi + 1) * 128, h * VD:(h + 1) * VD],
                        in_=ot,
                    )
    nc.compile()
    return nc


_NC_CACHE = None


def _get_nc():
    global _NC_CACHE
    if _NC_CACHE is None:
        _NC_CACHE = _build_program()
    return _NC_CACHE


def _shard_inputs(q, latent_vec_k, W_kv_b):
    q4 = np.asarray(q, np.float32).reshape(B, S, H, QK)
    lat = np.asarray(latent_vec_k, np.float32)
    W4 = np.asarray(W_kv_b, np.float32).reshape(KV_RANK, H, NOPE + VD)
    tri = np.triu(np.ones((128, 128), np.float32)).astype(np.float16)

    in_maps = []
    for c in range(NCORES):
        b, g = divmod(c, NBG)
        hg = slice(HL * g, HL * g + HL)
        qh = q4[b, :, hg, :]  # [S, HL, QK]
        in_maps.append({
            "qtn": np.ascontiguousarray(qh[:, :, :NOPE].transpose(1, 2, 0)).astype(np.float16),
            "qtr": np.ascontiguousarray(qh[:, :, NOPE:].transpose(1, 2, 0)).astype(np.float16),
            "latt": np.ascontiguousarray(lat[b * S:(b + 1) * S, :].T).astype(np.float16),
            "wk": W4[:, hg, :NOPE].reshape(KV_RANK, HL * NOPE).astype(np.float16),
            "wv": W4[:, hg, NOPE:].reshape(KV_RANK, HL * VD).astype(np.float16),
            "tri": tri,
        })
    return in_maps


def run_cores(q, latent_vec_k, W_kv_b, trace=False, **kw):
    nc = _get_nc()
    in_maps = _shard_inputs(q, latent_vec_k, W_kv_b)
    res = bass_utils.run_bass_kernel_spmd(
        nc, in_maps, core_ids=list(range(NCORES)), trace=trace, **kw
    )
    full = np.empty((B * S, H * VD), np.float32)
    for c in range(NCORES):
        b, g = divmod(c, NBG)
        full[b * S:(b + 1) * S, g * HL * VD:(g + 1) * HL * VD] = res.results[c]["out"]
    return full, res


def kernel(q, latent_vec_k, W_kv_b):
    return run_cores(q, latent_vec_k, W_kv_b)[0]
